# revision 13
# baseline (speedup 1.0000x reference)
"""Causal single-head attention (B=4, S=4096, D=1024, fp32) on 8 TRN2 NeuronCores.

Sharding: data-parallel over batch (4) x 2-way causal-balanced query split.
Core c handles batch c//2; role r = c%2 takes global 512-row query blocks
[1,3,5,7] (r=1) or [0,2,4,6] (r=0), assigned to 4 "slots" with uniform
per-slot key-chunk capacities [8,16,24,32] so all 8 cores run one SPMD
program; causality and per-core block offsets are enforced purely by data
(mask thresholds DMA'd per core). No collectives (measured slower than the
duplicated half of the k/v projection they would remove).

Per-core pipeline (all matmuls on TensorE, fp32r projections ~1e-4 rel err):
  1) v = x @ Wv -> bf16, spilled to DRAM; kT = (x@Wk).T and qT = (x@Wq).T
     -> bf16, SBUF-resident. Weights double-buffered so each 4MB weight DMA
     hides under the previous projection's matmuls.
  2) per slot: scoresT[key,q] = kT-chunks.T @ qT (bf16), exp on ScalarE
     (scale 1/32) into a bf16 strip, causal mask = (iota >= thr) on VectorE,
     denominators via ones-matmul column sums, out.T[e,q] accumulated in
     PSUM over key chunks, normalized by reciprocal(sums), DMA'd out.
Host transposes x and assembles the output.
"""
import sys
import numpy as np

sys.path.insert(0, "/opt/trn_rl_repo")

B, S, D = 4, 4096, 1024
P = 128
QB = 512
DC = D // P            # 8 contraction chunks of 128
NSLOT = 4
MAXKC = S // P         # 32
CAPS = [8, 16, 24, 32]
SKIPS = [0, 8, 16, 24]
QBLOCKS = [[0, 2, 4, 6], [1, 3, 5, 7]]   # role -> global 512-block per slot
NCORES = 8
QLOC = NSLOT * QB      # 2048 query rows per core
SCALE = 1.0 / np.sqrt(np.float32(D))     # softmax 1/sqrt(d_out)

_built = None


def _build():
    import concourse.mybir as mybir
    import concourse.tile as tile
    from concourse import bacc

    f32 = mybir.dt.float32
    bf16 = mybir.dt.bfloat16
    f32r = mybir.dt.float32r

    nc = bacc.Bacc("TRN2", target_bir_lowering=False, debug=False,
                   num_devices=NCORES)
    xT = nc.dram_tensor("xT", [D, S], f32r, kind="ExternalInput")
    xTq = nc.dram_tensor("xTq", [D, QLOC], f32r, kind="ExternalInput")
    Wq = nc.dram_tensor("Wq", [D, D], f32r, kind="ExternalInput")
    Wk = nc.dram_tensor("Wk", [D, D], f32r, kind="ExternalInput")
    Wv = nc.dram_tensor("Wv", [D, D], f32r, kind="ExternalInput")
    thr = nc.dram_tensor("thr", [P, NSLOT * MAXKC], f32, kind="ExternalInput")
    iota = nc.dram_tensor("iota", [P, QB], f32, kind="ExternalInput")
    outT = nc.dram_tensor("outT", [D, QLOC], f32, kind="ExternalOutput")

    xT_r = xT.ap().rearrange("(c p) s -> p c s", p=P)
    xTq_r = xTq.ap().rearrange("(c p) s -> p c s", p=P)
    W_r = {"q": Wq.ap().rearrange("(c p) e -> p c e", p=P),
           "k": Wk.ap().rearrange("(c p) e -> p c e", p=P),
           "v": Wv.ap().rearrange("(c p) e -> p c e", p=P)}

    with tile.TileContext(nc) as tc, \
         tc.tile_pool(name="res", bufs=1) as res, \
         tc.tile_pool(name="const", bufs=1) as constp, \
         tc.tile_pool(name="p1small", bufs=3) as p1small, \
         tc.tile_pool(name="dram", bufs=1, space="DRAM") as dramp, \
         tc.tile_pool(name="psA", bufs=4, space="PSUM") as psA, \
         tc.tile_pool(name="psS", bufs=3, space="PSUM") as psS, \
         tc.tile_pool(name="psR", bufs=1, space="PSUM") as psR:

        kT = res.tile([P, DC, S], bf16, tag="kT")
        qT = res.tile([P, DC, QLOC], bf16, tag="qT")
        vsp = dramp.tile([S, D], bf16, tag="vsp")

        iota_sb = constp.tile([P, QB], f32, tag="iota")
        thr_sb = constp.tile([P, NSLOT * MAXKC], f32, tag="thr")
        ones_sb = constp.tile([P, P], bf16, tag="ones")
        nc.sync.dma_start(out=iota_sb[:], in_=iota.ap())
        nc.sync.dma_start(out=thr_sb[:], in_=thr.ap())
        nc.vector.memset(ones_sb[:], 1.0)

        # ---------------- phase 1: projections (fp32r) ----------------
        # Order: v (Wv) -> kT (Wk) -> qT (Wq). Two weight slots so the next
        # weight's DMA hides under the current projection's matmuls. Weight
        # and x-strip DMAs are split per 128-d chunk so the first matmul
        # only waits for one chunk.
        with tc.tile_pool(name="wa", bufs=1) as wa, \
             tc.tile_pool(name="wb", bufs=1) as wb, \
             tc.tile_pool(name="xs", bufs=2) as xs:

            def load_w(pool, which, nm):
                w_sb = pool.tile([P, DC, D], f32r, tag=pool.name, name=nm)
                for dc in range(DC):
                    nc.sync.dma_start(out=w_sb[:, dc], in_=W_r[which][:, dc])
                return w_sb

            def load_xstrip(src_r, blk, nm):
                xstrip = xs.tile([P, DC, QB], f32r, tag="xs", name=nm)
                for dc in range(DC):
                    nc.sync.dma_start(
                        out=xstrip[:, dc],
                        in_=src_r[:, dc, blk * QB:(blk + 1) * QB])
                return xstrip

            wv_sb = load_w(wa, "v", "wv_sb")
            wk_sb = None

            # Warm the PE HAM clock-gate with throwaway matmuls while the
            # first weight/x DMAs are in flight (ones_sb only needs the
            # memset). ~5us of dummy work flips the PE to 2.4 GHz before
            # the real matmuls arrive.
            warm = psR.tile([P, P], f32, tag="sum", name="warm")
            for i in range(260):
                nc.tensor.matmul(warm[:], lhsT=ones_sb[:],
                                 rhs=ones_sb[:], start=True, stop=True)

            # v = x @ Wv -> vsp (s-major bf16, spilled to DRAM)
            for blk in range(S // QB):
                xstrip = load_xstrip(xT_r, blk, f"xv_{blk}")
                if blk == 2:
                    # defer the Wk DMA so Wv + the lead x-strips get the
                    # full DMA bandwidth at kernel start
                    wk_sb = load_w(wb, "k", "wk_sb")
                for ss in range(QB // P):
                    for eb in range(D // QB):
                        acc = psA.tile([P, QB], f32, tag="acc",
                                       name=f"vacc_{blk}_{ss}_{eb}")
                        for dc in range(DC):
                            nc.tensor.matmul(
                                acc[:],
                                lhsT=xstrip[:, dc, ss * P:(ss + 1) * P],
                                rhs=wv_sb[:, dc, eb * QB:(eb + 1) * QB],
                                start=(dc == 0), stop=(dc == DC - 1))
                        vtmp = p1small.tile([P, QB], bf16, tag="vtmp",
                                            name=f"vtmp_{blk}_{ss}_{eb}")
                        if (ss + eb) % 2 == 0:
                            nc.vector.tensor_copy(vtmp[:], acc[:])
                        else:
                            nc.scalar.copy(vtmp[:], acc[:])
                        r0 = blk * QB + ss * P
                        nc.sync.dma_start(
                            out=vsp[r0:r0 + P, eb * QB:(eb + 1) * QB],
                            in_=vtmp[:])

            # kT = (x @ Wk).T ; Wq prefetches into Wv's slot meanwhile
            wq_sb = load_w(wa, "q", "wq_sb")

            def proj_T(dst, src_r, nblocks, w_sb, pfx):
                for blk in range(nblocks):
                    xstrip = load_xstrip(src_r, blk, f"{pfx}x_{blk}")
                    for ec in range(DC):
                        acc = psA.tile([P, QB], f32, tag="acc",
                                       name=f"{pfx}acc_{blk}_{ec}")
                        for dc in range(DC):
                            nc.tensor.matmul(
                                acc[:],
                                lhsT=w_sb[:, dc, ec * P:(ec + 1) * P],
                                rhs=xstrip[:, dc],
                                start=(dc == 0), stop=(dc == DC - 1))
                        d = dst[:, ec, blk * QB:(blk + 1) * QB]
                        if ec % 2 == 0:
                            nc.vector.tensor_copy(d, acc[:])
                        else:
                            nc.scalar.copy(d, acc[:])

            proj_T(kT, xT_r, S // QB, wk_sb, "k")
            proj_T(qT, xTq_r, QLOC // QB, wq_sb, "q")

        # ---------------- phase 2: attention ----------------
        with tc.tile_pool(name="expp", bufs=2) as expp, \
             tc.tile_pool(name="vs", bufs=6) as vs, \
             tc.tile_pool(name="p2small", bufs=3) as p2s:
            for j in range(NSLOT):
                cap, skip = CAPS[j], SKIPS[j]
                expT = expp.tile([P, MAXKC, QB], bf16, tag="expT",
                                 name=f"expT_{j}")
                # scoresT -> exp -> mask
                for kc in range(cap):
                    sc = psS.tile([P, QB], f32, tag="sc",
                                  name=f"sc_{j}_{kc}")
                    for ec in range(DC):
                        nc.tensor.matmul(
                            sc[:],
                            lhsT=kT[:, ec, kc * P:(kc + 1) * P],
                            rhs=qT[:, ec, j * QB:(j + 1) * QB],
                            start=(ec == 0), stop=(ec == DC - 1))
                    nc.scalar.activation(
                        expT[:, kc], sc[:],
                        func=mybir.ActivationFunctionType.Exp,
                        scale=float(SCALE))
                    if kc >= skip:
                        m = p2s.tile([P, QB], bf16, tag="mask",
                                     name=f"m_{j}_{kc}")
                        nc.vector.tensor_scalar(
                            m[:], iota_sb[:],
                            thr_sb[:, j * MAXKC + kc:j * MAXKC + kc + 1],
                            None, mybir.AluOpType.is_ge)
                        nc.vector.tensor_mul(expT[:, kc], expT[:, kc], m[:])
                # denominators (column sums via ones-matmul)
                ds = psR.tile([P, QB], f32, tag="sum", name=f"ds_{j}")
                for kc in range(cap):
                    nc.tensor.matmul(ds[:], lhsT=ones_sb[:], rhs=expT[:, kc],
                                     start=(kc == 0), stop=(kc == cap - 1))
                sums_sb = p2s.tile([P, QB], f32, tag="sums",
                                   name=f"sums_{j}")
                nc.vector.tensor_copy(sums_sb[:], ds[:])
                recip = p2s.tile([P, QB], f32, tag="recip",
                                 name=f"recip_{j}")
                nc.vector.reciprocal(recip[:], sums_sb[:])
                # out.T accumulation, e in two halves of 4 chunks
                for half in range(2):
                    accs = [psA.tile([P, QB], f32, tag="acc",
                                     name=f"oacc_{j}_{half}_{i}")
                            for i in range(4)]
                    for kc in range(cap):
                        vh = vs.tile([P, QB], bf16, tag="vh",
                                     name=f"vh_{j}_{half}_{kc}")
                        nc.sync.dma_start(
                            out=vh[:],
                            in_=vsp[kc * P:(kc + 1) * P,
                                    half * QB:(half + 1) * QB])
                        for e4 in range(4):
                            nc.tensor.matmul(
                                accs[e4][:],
                                lhsT=vh[:, e4 * P:(e4 + 1) * P],
                                rhs=expT[:, kc],
                                start=(kc == 0), stop=(kc == cap - 1))
                    for e4 in range(4):
                        ot = p2s.tile([P, QB], f32, tag="ot",
                                      name=f"ot_{j}_{half}_{e4}")
                        nc.vector.tensor_mul(ot[:], accs[e4][:], recip[:])
                        r0 = (half * 4 + e4) * P
                        nc.sync.dma_start(
                            out=outT.ap()[r0:r0 + P, j * QB:(j + 1) * QB],
                            in_=ot[:])

    nc.finalize()
    return nc


def _get_nc():
    global _built
    if _built is None:
        _built = _build()
    return _built


def _host_inputs(x, Wq, Wk, Wv):
    iota = np.broadcast_to(
        np.arange(QB, dtype=np.float32), (P, QB)).copy()
    Wq = np.ascontiguousarray(np.asarray(Wq, dtype=np.float32))
    Wk = np.ascontiguousarray(np.asarray(Wk, dtype=np.float32))
    Wv = np.ascontiguousarray(np.asarray(Wv, dtype=np.float32))
    p = np.arange(P, dtype=np.float32)
    thrs = []
    for role in range(2):
        t = np.zeros((P, NSLOT * MAXKC), np.float32)
        for j in range(NSLOT):
            q0 = QBLOCKS[role][j] * QB
            for kc in range(MAXKC):
                t[:, j * MAXKC + kc] = np.clip(kc * P + p - q0, 0, QB)
        thrs.append(t)
    xTs = [np.ascontiguousarray(np.asarray(x[b]).T.astype(np.float32))
           for b in range(B)]
    in_maps = []
    for c in range(NCORES):
        b, role = divmod(c, 2)
        cols = np.concatenate(
            [np.arange(QBLOCKS[role][j] * QB, QBLOCKS[role][j] * QB + QB)
             for j in range(NSLOT)])
        xTq = np.ascontiguousarray(xTs[b][:, cols])
        in_maps.append({"xT": xTs[b], "xTq": xTq, "Wq": Wq, "Wk": Wk,
                        "Wv": Wv, "thr": thrs[role], "iota": iota})
    return in_maps


def _assemble(results):
    out = np.empty((B, S, D), np.float32)
    for c in range(NCORES):
        b, role = divmod(c, 2)
        oT = results[c]["outT"]
        for j in range(NSLOT):
            q0 = QBLOCKS[role][j] * QB
            out[b, q0:q0 + QB, :] = oT[:, j * QB:(j + 1) * QB].T
    return out


def run_cores(in_maps, trace=False):
    from concourse.bass_utils import run_bass_kernel_spmd
    nc = _get_nc()
    return run_bass_kernel_spmd(nc, in_maps, list(range(NCORES)), trace=trace)


def kernel(x, Wq, Wk, Wv):
    in_maps = _host_inputs(x, Wq, Wk, Wv)
    res = run_cores(in_maps, trace=False)
    return _assemble(res.results)
